# revision 11
# baseline (speedup 1.0000x reference)
"""Causal self-attention (B=2, T=2048, C=1024, H=16) on 8 TRN2 NeuronCores.

Sharding: tensor-parallel over heads (2 heads/core) for QKV + attention;
AllToAll re-shards to sequence-parallel for the output projection.

v3: one streaming pipeline. Attention chunks run in ascending-jl order with
batches alternating, so chunk (b, jl) only needs QKV of batch b up to row
512*(jl+1). The next QKV chunk is emitted as a *feeder* -- one ~1.2us step
after each attention kt-group -- which keeps the PE instruction stream dense
(no idle windows, HAM stays at the warm 2.4 GHz clock) and fully overlaps
QKV with attention. Each chunk's softmax normalization + AllToAll is
deferred into the next chunk; output-projection pairs are emitted at chunk
boundaries once their exchanges completed, so only the last pair's exchange
is exposed at the end.

PSUM budget (8 banks): scores tag "s" [128,2,512] x 2 bufs (4 banks),
y [65,2,512] (2 banks), shared tag "qk" [128,2,512] (2 banks) time-shared
by QKV accumulation, the reciprocal-broadcast, and output-projection tiles.
"""
import os
import math
import threading

import numpy as np
import ml_dtypes

import concourse.bass as bass
import concourse.tile as tile
from concourse import mybir, bacc, bass_utils

B, T, C, H = 2, 2048, 1024, 16
D = C // H                 # 64
NCORES = 8
HPC = H // NCORES          # heads per core = 2
HC = HPC * D               # head-channels per core = 128
BT = B * T                 # 4096
TQ = 512                   # query chunk
TKT = 128                  # key tile
NCH = BT // TQ             # 8 chunks over B*T
SPC = TQ // D              # 8 strips of 64 rows per chunk (one per core)
ROWS = BT // NCORES        # output rows per core = 512
KT = C // 128              # 8 contraction tiles over channels
SM_SCALE = 1.0 / math.sqrt(D)

F32 = mybir.dt.float32
BF16 = mybir.dt.bfloat16
BF16_NP = ml_dtypes.bfloat16


def _build_program():
    nc = bacc.Bacc("TRN2", target_bir_lowering=False, debug=False,
                   num_devices=NCORES)
    xt = nc.dram_tensor("xt", [C, BT], BF16, kind="ExternalInput").ap()
    wqkv = nc.dram_tensor("wqkv", [C, 3 * HC], BF16, kind="ExternalInput").ap()
    wproj = nc.dram_tensor("wproj", [C, C], BF16, kind="ExternalInput").ap()
    bq = nc.dram_tensor("bq", [HC, 1], F32, kind="ExternalInput").ap()
    bk = nc.dram_tensor("bk", [HC, 1], F32, kind="ExternalInput").ap()
    bv = nc.dram_tensor("bv", [1, HC], BF16, kind="ExternalInput").ap()
    bproj = nc.dram_tensor("bproj", [1, C], BF16, kind="ExternalInput").ap()
    maskt = nc.dram_tensor("maskt", [TKT, TKT], BF16, kind="ExternalInput").ap()
    outp = nc.dram_tensor("out", [ROWS, C], BF16, kind="ExternalOutput").ap()

    with tile.TileContext(nc) as tc:
        with (
            tc.tile_pool(name="consts", bufs=1) as consts,
            tc.tile_pool(name="xpool", bufs=2) as xpool,
            tc.tile_pool(name="ppool", bufs=3) as ppool,
            tc.tile_pool(name="npool", bufs=2) as npool,
            tc.tile_pool(name="ytpool", bufs=2) as ytpool,
            tc.tile_pool(name="opool", bufs=2) as opool,
            tc.tile_pool(name="dram", bufs=1, space="DRAM") as dram,
            tc.tile_pool(name="ps_s", bufs=2, space="PSUM") as ps_s,
            tc.tile_pool(name="ps_y", bufs=1, space="PSUM") as ps_y,
            tc.tile_pool(name="ps_qk", bufs=1, space="PSUM") as ps_qk,
        ):
            # ---- constants. xt chunk 0 leads the SP HWDGE ring; weights and
            # small tensors ride the ACT HWDGE ring in parallel; wproj (big,
            # needed last) takes the gpsimd SWDGE path ----
            def xt_load(c):
                tiles = []
                for kt in range(KT):
                    xx = xpool.tile([128, TQ], BF16, tag=f"xt{kt}",
                                    name=f"xt{c}_{kt}")
                    nc.sync.dma_start(
                        out=xx,
                        in_=xt[128 * kt:128 * (kt + 1), TQ * c:TQ * (c + 1)])
                    tiles.append(xx)
                return tiles

            xt0 = xt_load(0)
            wqkv_sb = []
            for kt in range(KT):
                w1 = consts.tile([128, 3 * HC], BF16, name=f"wqkv_sb{kt}")
                nc.scalar.dma_start(out=w1, in_=wqkv[128 * kt:128 * (kt + 1), :])
                wqkv_sb.append(w1)
            bq_sb = consts.tile([HC, 1], F32, name="bq_sb")
            nc.scalar.dma_start(out=bq_sb, in_=bq)
            bk_sb = consts.tile([HC, 1], F32, name="bk_sb")
            nc.scalar.dma_start(out=bk_sb, in_=bk)
            bv_sb = consts.tile([1, HC], BF16, name="bv_sb")
            nc.scalar.dma_start(out=bv_sb, in_=bv)
            bproj_sb = consts.tile([1, C], BF16, name="bproj_sb")
            nc.scalar.dma_start(out=bproj_sb, in_=bproj)
            mask_sb = consts.tile([TKT, TKT], BF16, name="mask_sb")
            nc.scalar.dma_start(out=mask_sb, in_=maskt)
            wproj_sb = []
            for kt in range(KT):
                w2 = consts.tile([128, C], BF16, name=f"wproj_sb{kt}")
                nc.gpsimd.dma_start(out=w2, in_=wproj[128 * kt:128 * (kt + 1), :])
                wproj_sb.append(w2)
            ones_bf = consts.tile([1, 128], BF16, name="ones_bf")
            nc.vector.memset(ones_bf, 1.0)

            qT_b = [consts.tile([HC, T], BF16, name=f"qT_sb{b}")
                    for b in range(B)]
            kT_b = [consts.tile([HC, T], BF16, name=f"kT_sb{b}")
                    for b in range(B)]
            v_sb = [consts.tile([128, HPC, D + 1], BF16, name=f"v_sb{tt}")
                    for tt in range(BT // 128)]
            for vt in v_sb:
                nc.vector.memset(vt[:, :, D:D + 1], 1.0)

            # force the exp table-set load early (ACT is free at the start)
            dummy = consts.tile([1, 1], F32, name="dummy_exp")
            nc.scalar.activation(out=dummy, in_=ones_bf[0:1, 0:1],
                                 func=mybir.ActivationFunctionType.Exp)

            a2a_in = [dram.tile([NCORES, HC, D], BF16, name=f"a2a_in{c}")
                      for c in range(NCH)]
            a2a_out = [dram.tile([NCORES, HC, D], BF16, name=f"a2a_out{c}")
                       for c in range(NCH)]

            # ---- QKV for global chunk c as a list of feeder steps.
            # q lands in qk-slot bank 0, k in bank 1; v strip s reuses
            # bank s%2 columns [128*(s//2), +128) after the q/k evacuations.
            def qkv_steps(c, xt_t):
                b, jl = c // (NCH // B), c % (NCH // B)
                state = {}

                def step_q():
                    qk = ps_qk.tile([128, HPC, TQ], F32, tag="qk",
                                    name=f"qk{c}")
                    state["qk"] = qk
                    for kt in range(KT):
                        nc.tensor.matmul(qk[:, 0, :],
                                         lhsT=wqkv_sb[kt][:, 0:HC],
                                         rhs=xt_t[kt],
                                         start=(kt == 0), stop=(kt == KT - 1))
                    nc.vector.tensor_scalar(
                        out=qT_b[b][:, TQ * jl:TQ * (jl + 1)], in0=qk[:, 0, :],
                        scalar1=bq_sb, scalar2=None, op0=mybir.AluOpType.add)

                def step_k():
                    qk = state["qk"]
                    for kt in range(KT):
                        nc.tensor.matmul(qk[:, 1, :],
                                         lhsT=wqkv_sb[kt][:, HC:2 * HC],
                                         rhs=xt_t[kt],
                                         start=(kt == 0), stop=(kt == KT - 1))
                    nc.vector.tensor_scalar(
                        out=kT_b[b][:, TQ * jl:TQ * (jl + 1)], in0=qk[:, 1, :],
                        scalar1=SM_SCALE, scalar2=bk_sb,
                        op0=mybir.AluOpType.mult, op1=mybir.AluOpType.add)

                def make_step_v(s0):
                    def step():
                        qk = state["qk"]
                        for s in (s0, s0 + 1):
                            tt = 4 * c + s
                            off = 128 * (s // 2)
                            reg = (qk[:, s % 2, off:off + 128]
                                   .rearrange("p (h x) -> p h x", h=HPC))
                            for kt in range(KT):
                                nc.tensor.matmul(
                                    reg,
                                    lhsT=xt_t[kt][:, 128 * s:128 * (s + 1)],
                                    rhs=wqkv_sb[kt][:, 2 * HC:3 * HC],
                                    start=(kt == 0), stop=False)
                            nc.tensor.matmul(reg, lhsT=ones_bf, rhs=bv_sb,
                                             start=False, stop=True)
                            nc.vector.tensor_copy(out=v_sb[tt][:, :, 0:D],
                                                  in_=reg)
                    return step

                return [step_q, step_k, make_step_v(0), make_step_v(2)]

            # ---- output projection for a chunk pair, two feeder steps;
            # po shares the qk PSUM slot (bank n per output half) ----
            def stage4_steps(cA, cB):
                state = {}

                def half(n):
                    if n == 0:
                        yy = opool.tile([128, KT, 2, D], BF16, tag="yy",
                                        name=f"yy{cA}")
                        state["yy"] = yy
                        state["po"] = ps_qk.tile([128, HPC, TQ], F32,
                                                 tag="qk", name=f"po{cA}")
                        for ci, cc in enumerate((cA, cB)):
                            nc.sync.dma_start(
                                out=yy[:, :, ci, :],
                                in_=a2a_out[cc].rearrange("k p q -> p k q"))
                    yy, po = state["yy"], state["po"][:, n, :]
                    for kt in range(KT):
                        nc.tensor.matmul(
                            po, lhsT=yy[:, kt, :, :],
                            rhs=wproj_sb[kt][:, TQ * n:TQ * (n + 1)],
                            start=(kt == 0), stop=False)
                    nc.tensor.matmul(
                        po, lhsT=ones_bf[0:1, 0:128],
                        rhs=bproj_sb[0:1, TQ * n:TQ * (n + 1)],
                        start=False, stop=True)
                    osb = opool.tile([128, TQ], BF16, tag="osb")
                    nc.vector.tensor_copy(out=osb, in_=po)
                    for ci, cc in enumerate((cA, cB)):
                        nc.sync.dma_start(
                            out=outp[D * cc:D * (cc + 1),
                                     TQ * n:TQ * (n + 1)],
                            in_=osb[D * ci:D * (ci + 1), :])

                return [lambda: half(0), lambda: half(1)]

            # ---- the streaming schedule ----
            proc = [(b, jl) for jl in range(NCH // B) for b in range(B)]
            # processed cidx order: 0, 4, 1, 5, 2, 6, 3, 7
            seq = [(NCH // B) * b + jl for (b, jl) in proc]
            feed_order = seq[1:]            # QKV chunk fed during att #i-1
            pending = [None]
            feeder = []

            def make_tail(yc, recb, cidx):
                def tail():
                    rbc_t = ps_qk.tile([128, HPC, TQ], F32, tag="qk",
                                       name=f"rbc{cidx}")
                    for h in range(HPC):
                        nc.tensor.matmul(
                            rbc_t[0:D, h, :], lhsT=ones_bf[0:1, 0:D],
                            rhs=recb[0:1, h, :], start=True, stop=True)
                    yt = ytpool.tile([D, HPC, TQ], BF16, tag="yt")
                    nc.vector.tensor_mul(yt, yc, rbc_t[0:D, :, :])
                    for h in range(HPC):
                        nc.sync.dma_start(
                            out=a2a_in[cidx][:, D * h:D * (h + 1), :]
                                .rearrange("s p q -> p s q"),
                            in_=yt[:, h, :]
                                .rearrange("p (s q) -> p s q", s=SPC))
                    nc.gpsimd.collective_compute(
                        "AllToAll", mybir.AluOpType.bypass,
                        replica_groups=[list(range(NCORES))],
                        ins=[a2a_in[cidx].opt()],
                        outs=[a2a_out[cidx].opt()])
                return tail

            def attention(i):
                b, jl = proc[i]
                cidx = seq[i]
                nkt = (TQ // TKT) * (jl + 1)
                q0 = TQ * jl
                y = ps_y.tile([D + 1, HPC, TQ], F32, tag="y", name=f"y{cidx}")
                for kt in range(nkt):
                    r = kt - (TQ // TKT) * jl
                    k0 = TKT * kt
                    ss = ps_s.tile([TKT, HPC, TQ], F32, tag="s")
                    for h in range(HPC):
                        hp = D * h
                        nc.tensor.matmul(
                            ss[:, h, :],
                            lhsT=kT_b[b][hp:hp + D, k0:k0 + TKT],
                            rhs=qT_b[b][hp:hp + D, q0:q0 + TQ],
                            start=True, stop=True)
                    if kt == 0 and pending[0] is not None:
                        pending[0]()
                        pending[0] = None
                    pt = ppool.tile([TKT, HPC, TQ], BF16, tag="pt")
                    c0 = TKT * r if r > 0 else 0
                    if c0 > 0:
                        nc.vector.memset(pt[:, :, 0:c0], 0.0)
                    nc.scalar.activation(
                        out=pt[:, :, c0:TQ], in_=ss[:, :, c0:TQ],
                        func=mybir.ActivationFunctionType.Exp)
                    if r >= 0:
                        m0 = TKT * r
                        for h in range(HPC):
                            nc.vector.tensor_mul(
                                pt[:, h, m0:m0 + TKT],
                                pt[:, h, m0:m0 + TKT], mask_sb)
                    vt = v_sb[(T // 128) * b + kt]
                    for h in range(HPC):
                        nc.tensor.matmul(
                            y[:, h, :], lhsT=vt[:, h, :], rhs=pt[:, h, :],
                            start=(kt == 0), stop=(kt == nkt - 1))
                    if feeder:
                        feeder.pop(0)()
                while feeder:
                    feeder.pop(0)()
                # normalization front half; frees the y banks immediately
                den = npool.tile([1, HPC, TQ], F32, tag="den")
                nc.vector.tensor_copy(out=den, in_=y[D:D + 1, :, :])
                recf = npool.tile([1, HPC, TQ], F32, tag="recf")
                nc.vector.reciprocal_approx_fast(out=recf, in_=den)
                recb = npool.tile([1, HPC, TQ], BF16, tag="recb")
                nc.vector.tensor_copy(out=recb, in_=recf)
                yc = ytpool.tile([D, HPC, TQ], BF16, tag="yc")
                nc.vector.tensor_copy(out=yc, in_=y[0:D, :, :])
                pending[0] = make_tail(yc, recb, cidx)

            # prologue: QKV chunk 0 runs undisturbed
            for step in qkv_steps(0, xt0):
                step()
            s4 = {3: stage4_steps(seq[0], seq[1]),
                  5: stage4_steps(seq[2], seq[3])}
            for i in range(NCH):
                if i < NCH - 1:
                    cnext = feed_order[i]
                    xt_t = xt_load(cnext)
                    feeder.extend(qkv_steps(cnext, xt_t))
                attention(i)
                if i in s4:
                    for step in s4[i]:
                        step()
            pending[0]()
            pending[0] = None
            for cA, cB in ((seq[4], seq[5]), (seq[6], seq[7])):
                for step in stage4_steps(cA, cB):
                    step()

    nc.compile()
    return nc


_lock = threading.Lock()
_cached_nc = None
last_results = None  # BassKernelResults of the most recent kernel() call


def _get_program():
    global _cached_nc
    with _lock:
        if _cached_nc is None:
            _cached_nc = _build_program()
    return _cached_nc


def _host_inputs(x, W_qkv, b_qkv, W_proj, b_proj):
    bf = lambda a: np.ascontiguousarray(a).astype(BF16_NP)
    x = np.asarray(x, dtype=np.float32)
    W_qkv = np.asarray(W_qkv, dtype=np.float32)
    b_qkv = np.asarray(b_qkv, dtype=np.float32)
    W_proj = np.asarray(W_proj, dtype=np.float32)
    b_proj = np.asarray(b_proj, dtype=np.float32)

    xt = bf(x.reshape(BT, C).T)                     # [C, BT]
    wproj = bf(W_proj)                              # [C, C]
    bproj = bf(b_proj.reshape(1, C))
    k = np.arange(TKT)[:, None]
    q = np.arange(TKT)[None, :]
    maskt = (k <= q).astype(BF16_NP)                # [128, 128] tril boundary

    in_maps = []
    for i in range(NCORES):
        sel = slice(HC * i, HC * (i + 1))
        wq = W_qkv[:, sel]
        wk = W_qkv[:, C + HC * i:C + HC * (i + 1)]
        wv = W_qkv[:, 2 * C + HC * i:2 * C + HC * (i + 1)]
        in_maps.append({
            "xt": xt,
            "wqkv": bf(np.concatenate([wq, wk, wv], axis=1)),
            "wproj": wproj,
            "bq": np.ascontiguousarray(
                b_qkv[sel].reshape(HC, 1)).astype(np.float32),
            "bk": np.ascontiguousarray(
                (b_qkv[C + HC * i:C + HC * (i + 1)] * SM_SCALE)
                .reshape(HC, 1)).astype(np.float32),
            "bv": b_qkv[2 * C + HC * i:2 * C + HC * (i + 1)]
                .reshape(1, HC).astype(BF16_NP),
            "bproj": bproj,
            "maskt": maskt,
        })
    return in_maps


def kernel(x, W_qkv, b_qkv, W_proj, b_proj):
    global last_results
    nc = _get_program()
    in_maps = _host_inputs(x, W_qkv, b_qkv, W_proj, b_proj)
    trace = bool(int(os.environ.get("KERNEL_TRACE", "0")))
    res = bass_utils.run_bass_kernel_spmd(
        nc, in_maps, core_ids=list(range(NCORES)), trace=trace)
    last_results = res
    # core s's output rows are strip s (64 rows) of every 512-row chunk
    arr = np.stack([np.asarray(res.results[s]["out"])
                    .astype(np.float32).reshape(BT // TQ, D, C)
                    for s in range(NCORES)], axis=1)   # [chunk, core, 64, C]
    return np.ascontiguousarray(arr.reshape(B, T, C))
